# revision 10
# baseline (speedup 1.0000x reference)
"""Trainium2 Bass kernel for nn_DiffHistogram (Gaussian soft-binned histogram).

Computes, for x of shape [B=8, C=8, H=256, W=256] and 32 bin centers:
    out[b, c*32+k, 0, 0] = sum_{h,w} (ER/RATIO) * exp(-(clip(x)-c_k)^2 / (2*sigma^2))

Sharding: data-parallel over batch B across 8 NeuronCores; each core handles
one sample [C, H*W] and computes its full [C, 32] pooled histogram.

Per-core layout: SBUF tile [128, 4096] with partition p = (c*16 + g):
channel c in 0..7, pixel-group g in 0..15, 4096 pixels along free dim.

v2 pipeline — split the 32 bins between the ACT and DVE engines:
  * ACT computes N_A "anchor" bins directly (Derivative_Erf with per-bin
    bias, accum_out), ~3785 ns/bin. ACT is 1x for all dtypes; this is its
    floor.
  * Remaining bins hang off anchors via the multiplicative recurrence
        e_{k+1} = e_k * exp(1024*D*x) * rho_k,  rho_k = exp(-512*D*(c_k+c_{k+1}))
    (D = bin spacing). DVE runs these chains in bf16 where tensor ops get
    2x/4x perf modes:
      RAW mode (DH_RAW=1): one in-place 2-out InstTensorTensor per bin:
        E_a *= S with accum to acc[:,k], where S = exp(1024*D*x - 512*D)
        absorbs a constant K = exp(-512*D) per hop; the per-bin residue
        exp(-h*lnK - 512*D*h*(c_a+c_{a+h})) is corrected in the final
        [8,32] multiply. ~2256 ns/bin (2x).
      SAFE mode (DH_RAW=0): tensor_tensor (m = E*R) + tensor_scalar
        (E = m*rho_k, accum). ~3323 ns/bin.
    bf16 chains: per-pixel relative error ~0.5-2% with pixel-pseudorandom
    sign, which averages out in 65536-pixel bin sums (tolerance 2e-2 L2).
  Anchor underflow (bf16 min normal ~1e-38 at |x-c_a|>0.41) only zeroes
  chained weights < 1e-14 in truth — harmless.

Accumulators live in the DMA-loaded `w` tile (zero-initialized columns),
one region per engine, so ACT and DVE never co-write one tensor. Final:
PE matmul with block-ones lhsT reduces 16 groups/channel over both accum
regions -> psum [8,32]; a small DVE multiply applies the per-bin
correction * ER/RATIO * sqrt(pi)/2; DMA out.

Written in raw Bass (no TileContext), semaphore-ordered by hand.
"""

import contextlib
import hashlib
import math
import os

import numpy as np

import concourse.bass as bass
import concourse.mybir as mybir
from concourse.bass_utils import run_bass_kernel_spmd

B = 8
C = 8
HW = 256 * 256          # 65536 pixels per channel
NBINS = 32
G = 128 // C            # 16 partition groups per channel
FREE = HW // G          # 4096 pixels per partition

ER = 1.0
RATIO = 2.5066
SIGMA = 1.0 / NBINS
INV_2SIG2 = 1.0 / (2.0 * SIGMA * SIGMA)    # 512.0
SQ = math.sqrt(INV_2SIG2)                  # 22.627417

# Derivative_Erf(t) = 2/sqrt(pi) * exp(-t^2); anchors/chains carry the
# 2/sqrt(pi); the final correction multiplies it away along with ER/RATIO.
DERF_OUT_SCALE = (ER / RATIO) * (math.sqrt(math.pi) / 2.0)

N_ACT = int(os.environ.get("DH_NACT", "0") or 0)  # 0 -> per-mode default
# Chain-op flavor: "safe" = tensor_tensor + tensor_scalar (2 ops/bin, both
# with HW-supported semantics); "stt" = one scalar_tensor_tensor per bin
# (cost model says 1x, real HW tier unknown — measure); "raw" = 2-out
# InstTensorTensor (KNOWN BROKEN on HW: the accum output is silently
# dropped; kept for experiments only).
MODE = os.environ.get("DH_MODE", "") or (
    "raw" if os.environ.get("DH_RAW", "0") == "1" else "safe"
)
RAW = MODE == "raw"
XBF = os.environ.get("DH_XBF", "0") == "1"  # anchors read bf16 x copy

# w tile columns: [block-ones C | anchor biases 32 | corr 32 | accA 32 | accD 32]
COL_BIAS = C
COL_CORR = C + NBINS
COL_ACCA = C + 2 * NBINS
COL_ACCD = C + 3 * NBINS
COL_SBIAS = C + 4 * NBINS
W_COLS = C + 4 * NBINS + 1

_nc_cache: dict = {}
last_results = None


def _n_act():
    return N_ACT or {"raw": 12, "stt": 12, "safe": 15}[MODE]


def _plan(n_act: int):
    """Anchor positions and group sizes. Longer chains at low bins (small
    bin centers -> chain values shrink, no bf16 overflow)."""
    base, rem = NBINS // n_act, NBINS % n_act
    sizes = [base + 1] * rem + [base] * (n_act - rem)
    anchors, pos = [], 0
    for s in sizes:
        anchors.append(pos)
        pos += s
    return anchors, sizes


def _build(bin_centers: np.ndarray, reps: int = 1) -> "bass.Bass":
    n_act = _n_act()
    mode = MODE
    raw = RAW
    key = (reps, n_act, mode, XBF, tuple(np.asarray(bin_centers, np.float64).tolist()))
    if key in _nc_cache:
        return _nc_cache[key]

    bc = np.asarray(bin_centers, np.float64)
    D = float((bc[-1] - bc[0]) / (NBINS - 1))     # bin spacing (1/31)
    LNK = -512.0 * D                               # RAW per-hop constant
    anchors, sizes = _plan(n_act)
    max_h = max(sizes) - 1

    if raw and max_h > 0:
        # bf16 overflow guard: chained value exp(h*(512*D)*(c_a+c_{a+h}-1)),
        # times one extra S factor (<= e^17) transiently.
        worst = max(
            h * 512.0 * D * (bc[a] + bc[a + h] - 1.0)
            for a, s in zip(anchors, sizes)
            for h in range(1, s)
        )
        assert worst + 17.0 < 85.0, f"bf16 overflow risk: e^{worst + 17.0:.1f}"

    f32 = mybir.dt.float32
    bf16 = mybir.dt.bfloat16
    alu = mybir.AluOpType
    act_fn = mybir.ActivationFunctionType

    nc = bass.Bass("TRN2", target_bir_lowering=False, debug=False, num_devices=B)
    x_d = nc.dram_tensor("x", [C, HW], f32, kind="ExternalInput")
    w_d = nc.dram_tensor("w", [128, W_COLS], f32, kind="ExternalInput")
    # The neuron NEFF cache has been observed to key on a program-independent
    # HLO digest, serving stale binaries across different bass programs with
    # identical I/O signatures. A nonce input with a program-derived shape
    # makes every distinct build's HLO signature unique.
    nsz = 3 + int(hashlib.md5(repr(key).encode()).hexdigest(), 16) % 509
    nc.dram_tensor("nonce", [1, nsz], f32, kind="ExternalInput")
    out_d = nc.dram_tensor("out", [C, NBINS], f32, kind="ExternalOutput")

    with contextlib.ExitStack() as st:
        X = st.enter_context(nc.sbuf_tensor("X", [128, FREE], f32))
        Xb = st.enter_context(nc.sbuf_tensor("Xb", [128, FREE], bf16)) if XBF else None
        S = st.enter_context(nc.sbuf_tensor("S", [128, FREE], bf16))
        chain_ais = [ai for ai in range(n_act) if sizes[ai] > 1]
        slot_of = {ai: i for i, ai in enumerate(chain_ais)}
        n_scratch = min(2, n_act - len(chain_ais))
        n_slices = len(chain_ais) + n_scratch
        for i, ai in enumerate([a for a in range(n_act) if sizes[a] == 1]):
            slot_of[ai] = len(chain_ais) + (i % n_scratch) if n_scratch else 0
        E = st.enter_context(nc.sbuf_tensor("E", [128, n_slices * FREE], bf16))
        Ms = None
        if mode == "safe":
            Ms = st.enter_context(nc.sbuf_tensor("Ms", [128, 2 * FREE], bf16))
        wt = st.enter_context(nc.sbuf_tensor("wt", [128, W_COLS], f32))
        out_sb = st.enter_context(nc.sbuf_tensor("out_sb", [C, NBINS], f32))
        ps = st.enter_context(nc.psum_tensor("ps", [C, NBINS], f32))

        s_dmx = st.enter_context(nc.semaphore("s_dmx"))
        s_dmq2 = st.enter_context(nc.semaphore("s_dmq2"))
        s_dmq3 = st.enter_context(nc.semaphore("s_dmq3"))
        s_dmw = st.enter_context(nc.semaphore("s_dmw"))
        s_act = st.enter_context(nc.semaphore("s_act"))
        s_dve = st.enter_context(nc.semaphore("s_dve"))
        s_pe = st.enter_context(nc.semaphore("s_pe"))
        s_out = st.enter_context(nc.semaphore("s_out"))
        s_dma = st.enter_context(nc.semaphore("s_dma"))

        block = st.enter_context(nc.Block())

        def esl(ai):
            i = slot_of[ai]
            return E.ap()[:, i * FREE : (i + 1) * FREE]

        def acc_a(k):
            return wt.ap()[:, COL_ACCA + k : COL_ACCA + k + 1]

        def acc_d(k):
            return wt.ap()[:, COL_ACCD + k : COL_ACCD + k + 1]

        # --- schedule bookkeeping (values are 1-based sem targets) ---
        # ACT op order: [S pass] + reps * anchors
        act_idx_of = {}
        n = 1  # S pass
        for r in range(reps):
            for ai in range(n_act):
                n += 1
                act_idx_of[(r, ai)] = n
        total_act = n

        # DVE emission: anchor-pair-major so each anchor's chain finishes
        # early in the rep's stream (the next rep's ACT rewrite of that
        # anchor then unblocks early — level-major ordering serializes the
        # rep boundary). Within a pair, levels interleave the two chains
        # to hide the serial RAW dependency. RAW: one op per bin; SAFE:
        # tt,tt then ts,ts using Ms slots 0/1.
        # Pre-pass computes every op with its waits; the vector block
        # just emits.
        dve_ops = []       # (kind, r, ai, h, slot, waits=[(which, val)])
        dve_idx_of = {}    # (r, ai, h) -> sem value once the BIN is done
        n = 1 if XBF else 0  # op 1 = Xb convert
        slot_last_ts = [0, 0]
        for r in range(reps):
            for p0 in range(0, n_act, 2):
                pair_all = list(range(p0, min(p0 + 2, n_act)))
                ph = max(sizes[ai] for ai in pair_all)
                for h in range(1, ph):
                    live = [ai for ai in pair_all if sizes[ai] > h]
                    if mode in ("raw", "stt"):
                        for ai in live:
                            w = [("act", act_idx_of[(r, ai)])] if h == 1 else [
                                ("dve", dve_idx_of[(r, ai, h - 1)])
                            ]
                            n += 1
                            dve_idx_of[(r, ai, h)] = n
                            dve_ops.append((mode, r, ai, h, 0, w))
                    else:
                        tt_idx = {}
                        for m, ai in enumerate(live):
                            w = [("act", act_idx_of[(r, ai)])] if h == 1 else [
                                ("dve", dve_idx_of[(r, ai, h - 1)])
                            ]
                            if slot_last_ts[m] > 0:
                                w.append(("dve", slot_last_ts[m]))
                            n += 1
                            tt_idx[ai] = n
                            dve_ops.append(("tt", r, ai, h, m, w))
                        for m, ai in enumerate(live):
                            w = [("dve", tt_idx[ai])]
                            n += 1
                            dve_idx_of[(r, ai, h)] = n
                            slot_last_ts[m] = n
                            dve_ops.append(("ts", r, ai, h, m, w))
        total_dve = n

        @block.sync
        def _(sync):
            xr = x_d.ap().rearrange("c (g j) -> (c g) j", g=G)
            sync.dma_start(X.ap()[0:64, :], xr[0:64, :]).then_inc(s_dmx, 16)
            sync.dma_start(wt.ap(), w_d.ap()).then_inc(s_dmw, 16)
            sync.wait_ge(s_out, 1)
            sync.dma_start(out_d.ap(), out_sb.ap()).then_inc(s_dma, 16)

        @block.gpsimd
        def _(gp):
            xr = x_d.ap().rearrange("c (g j) -> (c g) j", g=G)
            gp.dma_start(X.ap()[64:96, :], xr[64:96, :]).then_inc(s_dmq2, 16)

        @block.scalar
        def _(scalar):
            xr = x_d.ap().rearrange("c (g j) -> (c g) j", g=G)
            scalar.dma_start(X.ap()[96:128, :], xr[96:128, :]).then_inc(s_dmq3, 16)
            scalar.wait_ge(s_dmx, 16)
            scalar.wait_ge(s_dmq2, 16)
            scalar.wait_ge(s_dmq3, 16)
            scalar.wait_ge(s_dmw, 16)
            # S = exp(1024*D*x + LNK) (RAW) or R = exp(1024*D*x) (SAFE)
            nc.scalar.activation(
                S.ap(), X.ap(), act_fn.Exp,
                scale=1024.0 * D,
                bias=wt.ap()[:, COL_SBIAS : COL_SBIAS + 1],
            ).then_inc(s_act, 1)
            if XBF:
                scalar.wait_ge(s_dve, 1)  # Xb conversion done
            xin = Xb.ap() if XBF else X.ap()
            slot_user = {}
            for r in range(reps):
                for ai, a in enumerate(anchors):
                    if r > 0 and sizes[ai] > 1:
                        # E slice & acc rewritten; DVE is done with the
                        # previous rep's chain off this anchor.
                        scalar.wait_ge(
                            s_dve, dve_idx_of[(r - 1, ai, sizes[ai] - 1)]
                        )
                    elif sizes[ai] == 1:
                        prev = slot_user.get(slot_of[ai])
                        if prev is not None:
                            scalar.wait_ge(s_act, prev)
                        slot_user[slot_of[ai]] = act_idx_of[(r, ai)]
                    nc.scalar.activation(
                        esl(ai), xin, act_fn.Derivative_Erf,
                        scale=SQ, bias=wt.ap()[:, COL_BIAS + a : COL_BIAS + a + 1],
                        accum_out=acc_a(a),
                    ).then_inc(s_act, 1)

        @block.vector
        def _(vector):
            if XBF:
                vector.wait_ge(s_dmx, 16)
                vector.wait_ge(s_dmq2, 16)
                vector.wait_ge(s_dmq3, 16)
                nc.vector.tensor_scalar(
                    Xb.ap(), X.ap(), 1.0, None, op0=alu.mult
                ).then_inc(s_dve, 1)
            for kind, r, ai, h, m, waits in dve_ops:
                a = anchors[ai]
                for which, val in waits:
                    vector.wait_ge(s_act if which == "act" else s_dve, val)
                if kind == "raw":
                    inst = mybir.InstTensorTensor(
                        name=nc.get_next_instruction_name(),
                        op=alu.mult,
                        ins=[
                            vector.lower_ap(esl(ai)),
                            vector.lower_ap(S.ap()),
                        ],
                        outs=[
                            vector.lower_ap(esl(ai)),
                            vector.lower_ap(acc_d(a + h)),
                        ],
                    )
                    vector.add_instruction(inst).then_inc(s_dve, 1)
                elif kind == "stt":
                    rho = math.exp(-512.0 * D * (bc[a + h - 1] + bc[a + h]))
                    nc.vector.scalar_tensor_tensor(
                        esl(ai), esl(ai), rho, S.ap(),
                        op0=alu.mult, op1=alu.mult,
                        accum_out=acc_d(a + h),
                    ).then_inc(s_dve, 1)
                elif kind == "tt":
                    nc.vector.tensor_tensor(
                        Ms.ap()[:, m * FREE : (m + 1) * FREE],
                        esl(ai), S.ap(), op=alu.mult,
                    ).then_inc(s_dve, 1)
                else:
                    rho = math.exp(-512.0 * D * (bc[a + h - 1] + bc[a + h]))
                    nc.vector.tensor_scalar(
                        esl(ai),
                        Ms.ap()[:, m * FREE : (m + 1) * FREE],
                        rho, None, op0=alu.mult, op1=alu.add,
                        accum_out=acc_d(a + h),
                    ).then_inc(s_dve, 1)
            vector.wait_ge(s_pe, 1)
            nc.vector.tensor_tensor(
                out_sb.ap(), ps.ap(),
                wt.ap()[0:C, COL_CORR : COL_CORR + NBINS], op=alu.mult,
            ).then_inc(s_out, 1)

        @block.tensor
        def _(tensor):
            tensor.wait_ge(s_dmw, 16)
            tensor.wait_ge(s_act, total_act)
            if total_dve:
                tensor.wait_ge(s_dve, total_dve)
            nc.tensor.matmul(
                ps.ap(), wt.ap()[:, :C],
                wt.ap()[:, COL_ACCA : COL_ACCA + NBINS],
                start=True, stop=False,
            )
            nc.tensor.matmul(
                ps.ap(), wt.ap()[:, :C],
                wt.ap()[:, COL_ACCD : COL_ACCD + NBINS],
                start=False, stop=True,
            ).then_inc(s_pe, 1)

    _nc_cache[key] = nc
    return nc


def _weights(bin_centers=None) -> np.ndarray:
    if bin_centers is None:
        bin_centers = np.linspace(0.0, 1.0, NBINS)
    bc = np.asarray(bin_centers, np.float64)
    D = float((bc[-1] - bc[0]) / (NBINS - 1))
    LNK = -512.0 * D
    anchors, sizes = _plan(_n_act())

    w = np.zeros((128, W_COLS), np.float32)
    for c in range(C):
        w[c * G : (c + 1) * G, c] = 1.0
    for k in range(NBINS):
        w[:, COL_BIAS + k] = np.float32(-SQ * bc[k])
    corr = np.full(NBINS, DERF_OUT_SCALE, np.float64)
    if RAW:
        for ai, a in enumerate(anchors):
            for h in range(1, sizes[ai]):
                corr[a + h] = DERF_OUT_SCALE * math.exp(
                    -h * LNK - 512.0 * D * h * (bc[a] + bc[a + h])
                )
    w[:C, COL_CORR : COL_CORR + NBINS] = corr[None, :].astype(np.float32)
    w[:, COL_SBIAS] = np.float32(LNK if RAW else 0.0)
    return w


def kernel(x: np.ndarray, bin_centers: np.ndarray) -> np.ndarray:
    global last_results
    x = np.ascontiguousarray(np.asarray(x), dtype=np.float32)
    bc = np.asarray(bin_centers, dtype=np.float32)
    assert x.shape == (B, C, 256, 256), x.shape
    assert bc.shape == (NBINS,), bc.shape

    nc = _build(bc.astype(np.float64))
    w = _weights(bc.astype(np.float64))
    nsz = next(
        a.tensor_shape[1]
        for a in nc.m.functions[0].allocations
        if getattr(a, "memorylocations", None)
        and a.memorylocations[0].name == "nonce"
    )
    nz = np.zeros((1, nsz), np.float32)
    in_maps = [
        {"x": x[b].reshape(C, HW), "w": w, "nonce": nz} for b in range(B)
    ]
    res = run_bass_kernel_spmd(nc, in_maps, list(range(B)))
    last_results = res
    outs = [np.asarray(res.results[b]["out"], np.float32) for b in range(B)]
    return np.stack(outs).reshape(B, C * NBINS, 1, 1)
